# revision 1
# baseline (speedup 1.0000x reference)
"""HeteroGraphAttentionEmbedding Bass/Trainium2 kernel (8 NeuronCores), v2.

Strategy (no collectives; dst-sharded so every softmax segment is core-local):
  * Edges of each type are dst-sorted and bucketed into 128-dst blocks; type a
    -> cores 0-3 (out_item), type b -> cores 4-7 (out_user).
  * Blocks are processed in per-core descending-edge-count order (host permutes
    all per-block inputs and un-permutes the output), so one SPMD program with
    per-iteration static chunk counts NCNT[i] = max over cores wastes almost
    no padding chunks.
  * Phase 1 computes K|V rows (x_src @ [Wk|Wv], biases dropped: bk cancels in
    softmax, bv folded into the skip bias) only for src nodes referenced by
    this core's edges (host packs/remaps), bf16 in DRAM.
  * Phase 1b computes Q (written to DRAM for per-edge indirect gather) and the
    skip term S = x_dst @ Ws + (bs + bv) (resident in SBUF).
  * Phase 2 per block: one batched indirect gather for K|V rows and one for Q
    rows (amortizes the ~1us SWDGE fixed cost), then per 128-edge chunk:
      - y = W_t*rel + b_t via a 5-row broadcast matmul (hi/lo bf16 splits)
      - te = Sin(2*pi*(y - round(y))) on ACT
      - kev = [te|msg] @ We + gathered K|V via identity-matmul PSUM accumulate
      - alpha via fused scalar_tensor_tensor with accum_out (per head)
      - exp(alpha) = (1+t)/(1-t), t = tanh(alpha/2): Tanh shares the ACT
        table set with Sin, so there is no per-chunk table reload (the
        baseline's Sin/Exp alternation reloaded ACT tables twice per chunk).
      - numerator/denominator scattered with a one-hot matmul; softmax
        normalization + skip at the node level.
"""

import math

import numpy as np
import ml_dtypes

import concourse.bass as bass
import concourse.mybir as mybir
import concourse.tile as tile
from concourse import bacc
from concourse.bass import IndirectOffsetOnAxis

BF16 = ml_dtypes.bfloat16
F32 = np.float32

H, C = 2, 128
HC = 256
D_IN = 256
D_T = 128
D_MSG = 128
RSQRT_C = float(1.0 / np.float32(np.sqrt(C)))
EPS = 1e-16
TWO_PI = float(2.0 * math.pi)
PHK_ON_ACT = True      # int-convert of y on the scalar engine
USE_MOD = False        # range-reduce via a single DVE mod op (needs hw fmod)


class Cfg:
    def __init__(self, NSP, ND, NCNT):
        self.NSP = NSP              # padded packed source-node count
        self.ND = ND                # padded dst-node count per core
        self.NCNT = list(NCNT)      # chunks per block iteration (static)
        self.NB = ND // 128
        assert len(self.NCNT) == self.NB
        self.TC = int(sum(self.NCNT))
        cb = [0]
        for n in self.NCNT:
            cb.append(cb[-1] + n)
        self.cbase = cb
        assert NSP % 128 == 0 and ND % 128 == 0


def declare_ios(nc, cfg):
    dt = mybir.dt
    ins = {}

    def inp(name, shape, dtype):
        ins[name] = nc.dram_tensor(name, shape, dtype, kind="ExternalInput").ap()

    inp("xsT", [D_IN, cfg.NSP], dt.bfloat16)
    inp("xdT", [D_IN, cfg.ND], dt.bfloat16)
    inp("kvw", [128, 2, 512], dt.bfloat16)
    inp("qsw", [128, 2, 512], dt.bfloat16)
    inp("qsb", [1, 512], dt.bfloat16)
    inp("ew", [128, 2, 256], dt.bfloat16)
    inp("wb5", [5, 128], dt.bfloat16)
    inp("rel5", [5, cfg.TC * 128], dt.bfloat16)
    inp("iotaF", [128, 128], dt.bfloat16)
    inp("idmat", [128, 128], dt.bfloat16)
    inp("onesr", [1, 128], dt.bfloat16)
    inp("kvidx", [128, cfg.TC * 8], dt.int16)
    inp("qidx", [128, cfg.TC * 8], dt.int16)
    inp("dloc", [128, cfg.TC], dt.float32)
    inp("msgT", [128, cfg.TC * 128], dt.bfloat16)
    out = nc.dram_tensor("out", [cfg.ND, HC], dt.float32, kind="ExternalOutput").ap()
    kvt = nc.dram_tensor("kvt", [cfg.NSP, 512], dt.bfloat16).ap()
    qt = nc.dram_tensor("qt", [cfg.ND, 256], dt.bfloat16).ap()
    return ins, out, kvt, qt


def build_kernel(ctx, tc, ins, out, kvt, qt, cfg):
    import os
    PHASES = os.environ.get("KPHASES", "12b")
    nc = tc.nc
    dt = mybir.dt
    alu = mybir.AluOpType
    act = mybir.ActivationFunctionType
    NSP, ND, NB, NCNT, cbase = cfg.NSP, cfg.ND, cfg.NB, cfg.NCNT, cfg.cbase

    xsT, xdT = ins["xsT"], ins["xdT"]
    kvw, qsw, qsb, ew = ins["kvw"], ins["qsw"], ins["qsb"], ins["ew"]
    wb5, rel5 = ins["wb5"], ins["rel5"]
    iotaF, idmat, onesr = ins["iotaF"], ins["idmat"], ins["onesr"]
    kvidx, qidx, dloc, msgT = ins["kvidx"], ins["qidx"], ins["dloc"], ins["msgT"]

    # ---------- resident constants & tables ----------
    cpool = ctx.enter_context(tc.tile_pool(name="consts", bufs=1))
    kvw_sb = cpool.tile([128, 2, 512], dt.bfloat16)
    nc.sync.dma_start(kvw_sb[:], kvw[:])
    qsw_sb = cpool.tile([128, 2, 512], dt.bfloat16)
    nc.sync.dma_start(qsw_sb[:], qsw[:])
    qsb_sb = cpool.tile([1, 512], dt.bfloat16)
    nc.sync.dma_start(qsb_sb[:], qsb[:])
    ew_sb = cpool.tile([128, 2, 256], dt.bfloat16)
    nc.sync.dma_start(ew_sb[:], ew[:])
    wb5_sb = cpool.tile([5, 128], dt.bfloat16)
    nc.sync.dma_start(wb5_sb[:], wb5[:])
    iota_sb = cpool.tile([128, 128], dt.bfloat16)
    nc.sync.dma_start(iota_sb[:], iotaF[:])
    id_sb = cpool.tile([128, 128], dt.bfloat16)
    nc.sync.dma_start(id_sb[:], idmat[:])
    ones_sb = cpool.tile([1, 128], dt.bfloat16)
    nc.sync.dma_start(ones_sb[:], onesr[:])
    kvidx_sb = cpool.tile([128, cfg.TC * 8], dt.int16)
    nc.sync.dma_start(kvidx_sb[:], kvidx[:])
    qidx_sb = cpool.tile([128, cfg.TC * 8], dt.int16)
    nc.sync.dma_start(qidx_sb[:], qidx[:])
    dloc_sb = cpool.tile([128, cfg.TC], dt.float32)
    nc.sync.dma_start(dloc_sb[:], dloc[:])
    st_sb = cpool.tile([128, NB * 256], dt.bfloat16)   # resident skip terms

    # ---------- phases 1 & 1b interleaved: K|V table + Q table + skip ----------
    NXT = NSP // 128
    XCH = 8
    XG = 7
    assert NB % XG == 0
    NKVG = -(-NXT // XCH)
    NQG = NB // XG
    with tc.tile_pool(name="kv_x", bufs=3) as xpool, \
         tc.tile_pool(name="kv_ps", bufs=2, space="PSUM") as kvpsum, \
         tc.tile_pool(name="kv_sb", bufs=4) as kvsb, \
         tc.tile_pool(name="qx", bufs=2) as xdp, \
         tc.tile_pool(name="q_ps", bufs=2, space="PSUM") as qpsum, \
         tc.tile_pool(name="q_sb", bufs=3) as qsbp:
        kv_store_tiles = []
        q_store_tiles = []

        def kv_group(gi):
            base = gi * XCH
            cnt = min(XCH, NXT - base)
            xs = xpool.tile([128, 2, XCH * 128], dt.bfloat16, tag="xs",
                            name="xst")
            nc.sync.dma_start(
                xs[:, :, : cnt * 128],
                xsT[:, base * 128 : (base + cnt) * 128].rearrange(
                    "(t p) c -> p t c", p=128),
            )
            sb = kvsb.tile([128, XCH, 512], dt.bfloat16, tag="kvsb",
                           name="kvsbt")
            for i0 in range(0, cnt, 2):
                np_ = min(2, cnt - i0)
                ps = kvpsum.tile([128, 2, 512], dt.float32, tag="kvps",
                                 name="kvpst")
                for j in range(np_):
                    i = i0 + j
                    nc.tensor.matmul(
                        ps[:, j, :], lhsT=xs[:, 0, i * 128 : (i + 1) * 128],
                        rhs=kvw_sb[:, 0, :], start=True, stop=False,
                    )
                    nc.tensor.matmul(
                        ps[:, j, :], lhsT=xs[:, 1, i * 128 : (i + 1) * 128],
                        rhs=kvw_sb[:, 1, :], start=False, stop=True,
                    )
                # one wide strided copy per engine for the tile pair
                nc.vector.tensor_copy(sb[:, i0 : i0 + np_, 0:256],
                                      ps[:, :np_, 0:256])
                nc.scalar.copy(sb[:, i0 : i0 + np_, 256:512],
                               ps[:, :np_, 256:512])
            # one grouped store from the ACT queue (HWDGE issue cost is per
            # DMA instruction, so batching 8 tiles cuts it 8x)
            nc.scalar.dma_start(
                kvt[base * 128 : (base + cnt) * 128, :].rearrange(
                    "(g p) c -> p g c", p=128),
                sb[:, :cnt, :],
            )
            kv_store_tiles.append(sb)

        def q_group(qi):
            g = qi * XG
            xd = xdp.tile([128, 2, XG * 128], dt.bfloat16, tag="xd",
                          name="xdt")
            nc.sync.dma_start(
                xd[:],
                xdT[:, g * 128 : (g + XG) * 128].rearrange(
                    "(t p) c -> p t c", p=128),
            )
            qsb_t = qsbp.tile([128, XG, 256], dt.bfloat16, tag="qtile",
                              name="qtilet")
            for j in range(XG):
                i = g + j
                ps = qpsum.tile([128, 512], dt.float32, tag="qps",
                                name="qpst")
                nc.tensor.matmul(
                    ps[:], lhsT=xd[:, 0, j * 128 : (j + 1) * 128],
                    rhs=qsw_sb[:, 0, :], start=True, stop=False,
                )
                nc.tensor.matmul(
                    ps[:], lhsT=xd[:, 1, j * 128 : (j + 1) * 128],
                    rhs=qsw_sb[:, 1, :], start=False, stop=False,
                )
                nc.tensor.matmul(
                    ps[:], lhsT=ones_sb[:], rhs=qsb_sb[:], start=False,
                    stop=True,
                )
                nc.vector.tensor_copy(qsb_t[:, j, :], ps[:, 0:256])
                nc.scalar.copy(st_sb[:, i * 256 : (i + 1) * 256],
                               ps[:, 256:512])
            nc.scalar.dma_start(
                qt[g * 128 : (g + XG) * 128, :].rearrange(
                    "(j p) c -> p j c", p=128),
                qsb_t[:],
            )
            q_store_tiles.append(qsb_t)

        qi = 0
        for gi in range(NKVG):
            kv_group(gi)
            # interleave ~1 q-group per 2 kv-groups so PE/DVE/ACT/DMA of the
            # two production phases overlap instead of running back-to-back
            if gi % 2 == 1 and qi < NQG:
                q_group(qi)
                qi += 1
        while qi < NQG:
            q_group(qi)
            qi += 1

        # fence: gpsimd touches the store-source tiles, so the gpsimd queue
        # (which issues the phase-2 gathers next) cannot run until every
        # kvt/qt store DMA has completed -- makes gather-after-store ordering
        # explicit instead of relying on DRAM dependency tracking.
        for tfence in kv_store_tiles[-4:]:
            nc.gpsimd.memset(tfence[0:1, 0:1, 0:2], 0.0)
        for tfence in q_store_tiles[-3:]:
            nc.gpsimd.memset(tfence[0:1, 0:1, 0:2], 0.0)

    # ---------- phase 2: software-pipelined per-block attention ----------
    # Chunk-PAIR stages, one pair apart, so in-order engine queues never wait
    # on same-step producers; pairing halves per-op fixed overheads on the
    # te path, tanh, and the exp trio.
    if "2" not in PHASES:
        return
    KMAX = max(NCNT)
    pairs = []
    for i in range(NB):
        c = 0
        while c < NCNT[i]:
            n = min(2, NCNT[i] - c)
            pairs.append((i, c, n))
            c += n
    TP = len(pairs)

    with tc.tile_pool(name="bmsg", bufs=2) as msgpool, \
         tc.tile_pool(name="brel", bufs=2) as relpool, \
         tc.tile_pool(name="bkvg", bufs=2) as kvgpool, \
         tc.tile_pool(name="bqg", bufs=2) as qgpool, \
         tc.tile_pool(name="bsm", bufs=4) as smpool, \
         tc.tile_pool(name="bmv", bufs=4) as mvpool, \
         tc.tile_pool(name="bot", bufs=3) as otpool, \
         tc.tile_pool(name="bout", bufs=2) as outpool, \
         tc.tile_pool(name="ps_ob", bufs=1, space="PSUM") as obpsum, \
         tc.tile_pool(name="ps_kev", bufs=3, space="PSUM") as kevpsum, \
         tc.tile_pool(name="ps_acc", bufs=1, space="PSUM") as accpsum:

        blocks = {}
        state = {}
        groups = {}
        GRP = 7
        assert NB % GRP == 0
        MG = max(cbase[g + GRP] - cbase[g] for g in range(0, NB, GRP))

        def group_prologue(g0):
            kg = cbase[g0 + GRP] - cbase[g0]
            e0 = cbase[g0] * 128
            gr = {}
            gr["msg"] = msgpool.tile([128, MG * 128], dt.bfloat16, tag="msg",
                                     name="msgt")
            nc.sync.dma_start(gr["msg"][:, : kg * 128],
                              msgT[:, e0 : e0 + kg * 128])
            gr["rel"] = relpool.tile([5, MG * 128], dt.bfloat16, tag="rel",
                                     name="relt")
            nc.sync.dma_start(gr["rel"][:, : kg * 128],
                              rel5[:, e0 : e0 + kg * 128])
            gr["osb"] = outpool.tile([128, GRP, 256], dt.float32, tag="osb",
                                     name="osbt")
            groups[g0] = gr

        def prologue(i):
            k = NCNT[i]
            cb = cbase[i]
            if i % GRP == 0 and i + GRP < NB:
                group_prologue(i + GRP)
            b = {}
            g0 = (i // GRP) * GRP
            gr = groups[g0]
            off = (cb - cbase[g0]) * 128
            b["msg"] = gr["msg"][:, off : off + k * 128]
            b["rel"] = gr["rel"][:, off : off + k * 128]
            b["kvg"] = kvgpool.tile([128, KMAX, 512], dt.bfloat16, tag="kvg",
                                    name="kvgt")
            nc.gpsimd.dma_gather(
                b["kvg"][:, :k, :], kvt[:], kvidx_sb[:, cb * 8 : (cb + k) * 8],
                k * 128, k * 128, 512,
            )
            b["qg"] = qgpool.tile([128, KMAX, 256], dt.bfloat16, tag="qg",
                                  name="qgt")
            nc.gpsimd.dma_gather(
                b["qg"][:, :k, :], qt[:], qidx_sb[:, cb * 8 : (cb + k) * 8],
                k * 128, k * 128, 256,
            )
            ots = []
            for c in range(k):
                ot = otpool.tile([128, 128], dt.bfloat16, tag=f"ot{c}",
                                 name=f"ott{c}")
                nc.gpsimd.tensor_scalar(
                    out=ot[:], in0=iota_sb[:],
                    scalar1=dloc_sb[:, cb + c : cb + c + 1],
                    scalar2=None, op0=alu.is_equal,
                )
                ots.append(ot)
            b["ots"] = ots
            b["acc"] = accpsum.tile([128, 512], dt.float32, tag="acc",
                                    name="acct")
            blocks[i] = b

        def stage_t(t):
            i, c0, n = pairs[t]
            if c0 == 0 and i + 1 < NB:
                prologue(i + 1)
            b = blocks[i]
            w = n * 128
            ec = slice(c0 * 128, c0 * 128 + w)
            s = {}
            ob = obpsum.tile([128, 256], dt.float32, tag="ob", name="obt")
            nc.tensor.matmul(
                ob[:, :w], lhsT=wb5_sb[:], rhs=b["rel"][:, ec],
                start=True, stop=True,
            )
            phk = smpool.tile([128, 256], dt.int32, tag="phk", name="phkt")
            nc.scalar.copy(phk[:, :w], ob[:, :w])
            ph = smpool.tile([128, 256], dt.float32, tag="ph", name="pht")
            nc.vector.scalar_tensor_tensor(
                out=ph[:, :w], in0=ob[:, :w], scalar=0.0, in1=phk[:, :w],
                op0=alu.add, op1=alu.subtract,
            )
            te = smpool.tile([128, 256], dt.bfloat16, tag="te", name="tet")
            nc.scalar.activation(te[:, :w], ph[:, :w], act.Sin, scale=TWO_PI)
            s["te"] = te
            state[t] = s

        def stage_k(t):
            i, c0, n = pairs[t]
            b = blocks[i]
            s = state[t]
            te = s["te"]
            kev = kevpsum.tile([128, 2, 512], dt.float32, tag="kev",
                               name="kevt")
            for j in range(n):
                tj = te[:, j * 128 : (j + 1) * 128]
                mj = b["msg"][:, (c0 + j) * 128 : (c0 + j + 1) * 128]
                for half, kvs in ((0, slice(0, 256)), (1, slice(256, 512))):
                    o = kev[:, j, half * 256 : (half + 1) * 256]
                    nc.tensor.matmul(o, lhsT=tj, rhs=ew_sb[:, 0, :],
                                     start=True, stop=False)
                    nc.tensor.matmul(o, lhsT=mj, rhs=ew_sb[:, 1, :],
                                     start=False, stop=False)
                    nc.tensor.matmul(o, lhsT=id_sb[:],
                                     rhs=b["kvg"][:, c0 + j, kvs],
                                     start=False, stop=True)
            s["kev"] = kev

        def stage_b(t):
            i, c0, n = pairs[t]
            s = state[t]
            kev = s["kev"]
            alph = smpool.tile([128, 4], dt.float32, tag="alph", name="alpht")
            scr = smpool.tile([128, 128], dt.float32, tag="scr", name="scrt")
            for j in range(n):
                for h in range(2):
                    nc.vector.scalar_tensor_tensor(
                        out=scr[:],
                        in0=blocks[i]["qg"][:, c0 + j, h * 128 : (h + 1) * 128],
                        scalar=RSQRT_C,
                        in1=kev[:, j, h * 128 : (h + 1) * 128],
                        op0=alu.mult, op1=alu.mult,
                        accum_out=alph[:, 2 * j + h : 2 * j + h + 1],
                    )
            th = smpool.tile([128, 4], dt.float32, tag="th", name="tht")
            nc.scalar.activation(th[:, : 2 * n], alph[:, : 2 * n],
                                 act.Tanh, scale=0.5)
            s["th"] = th

        def stage_c(t):
            i, c0, n = pairs[t]
            k = NCNT[i]
            s = state.pop(t)
            kev, th = s["kev"], s["th"]
            b = blocks[i]
            w2 = 2 * n
            dn = smpool.tile([128, 4], dt.float32, tag="dn", name="dnt")
            nc.vector.tensor_scalar(out=dn[:, :w2], in0=th[:, :w2],
                                    scalar1=-1.0, scalar2=1.0,
                                    op0=alu.mult, op1=alu.add)
            rcp = smpool.tile([128, 4], dt.float32, tag="rcp", name="rcpt")
            nc.vector.reciprocal(rcp[:, :w2], dn[:, :w2])
            exx = smpool.tile([128, 4], dt.float32, tag="exx", name="exxt")
            nc.vector.scalar_tensor_tensor(
                out=exx[:, :w2], in0=th[:, :w2], scalar=1.0, in1=rcp[:, :w2],
                op0=alu.add, op1=alu.mult,
            )
            for j in range(n):
                c = c0 + j
                mv = mvpool.tile([128, 258], dt.bfloat16, tag="mv", name="mvt")
                if j == 0 and (c0 // 2) % 2 == 0:
                    nc.scalar.mul(mv[:, 0:128], kev[:, j, 256:384],
                                  exx[:, 2 * j : 2 * j + 1])
                else:
                    nc.vector.tensor_scalar(
                        out=mv[:, 0:128], in0=kev[:, j, 256:384],
                        scalar1=exx[:, 2 * j : 2 * j + 1],
                        scalar2=None, op0=alu.mult,
                    )
                nc.scalar.mul(mv[:, 128:256], kev[:, j, 384:512],
                              exx[:, 2 * j + 1 : 2 * j + 2])
                nc.gpsimd.tensor_copy(mv[:, 256:258],
                                       exx[:, 2 * j : 2 * j + 2])
                nc.tensor.matmul(
                    b["acc"][:, 0:258], lhsT=b["ots"][c][:], rhs=mv[:],
                    start=(c == 0), stop=(c == k - 1),
                )
            if c0 + n == k:
                acc = b["acc"]
                den = smpool.tile([128, 2], dt.float32, tag="den", name="dent")
                nc.vector.tensor_scalar(out=den[:], in0=acc[:, 256:258],
                                        scalar1=EPS, scalar2=None, op0=alu.add)
                rc2 = smpool.tile([128, 2], dt.float32, tag="rc2", name="rc2t")
                nc.vector.reciprocal(rc2[:], den[:])
                g0 = (i // GRP) * GRP
                osb = groups[g0]["osb"]
                jb = i - g0
                onrm = smpool.tile([128, 256], dt.float32, tag="onrm",
                                   name="onrmt")
                for h in range(2):
                    nc.scalar.mul(onrm[:, h * 128 : (h + 1) * 128],
                                  acc[:, h * 128 : (h + 1) * 128],
                                  rc2[:, h : h + 1])
                nc.gpsimd.tensor_tensor(
                    out=osb[:, jb, :], in0=onrm[:],
                    in1=st_sb[:, i * 256 : (i + 1) * 256], op=alu.add,
                )
                if jb == GRP - 1 or i == NB - 1:
                    nc.scalar.dma_start(
                        out[g0 * 128 : (g0 + jb + 1) * 128, :].rearrange(
                            "(b p) c -> p b c", p=128),
                        osb[:, : jb + 1, :],
                    )
                    del groups[g0]
                del blocks[i]

        group_prologue(0)
        prologue(0)
        for t in range(TP + 3):
            if t - 3 >= 0:
                stage_c(t - 3)
            if t < TP:
                stage_t(t)
            if 0 <= t - 1 < TP:
                stage_k(t - 1)
            if 0 <= t - 2 < TP:
                stage_b(t - 2)


def build_program(cfg):
    import contextlib
    import concourse.bacc as bacc_mod

    nc = bacc.Bacc("TRN2", target_bir_lowering=False, debug=False,
                   enable_asserts=False)
    ins, out, kvt, qt = declare_ios(nc, cfg)
    with tile.TileContext(nc) as tc:
        with contextlib.ExitStack() as ctx:
            build_kernel(ctx, tc, ins, out, kvt, qt, cfg)

    # All ACT functions used here (Sin, Tanh, Copy) live in the
    # silu_and_others table set; restrict the selection so the compiler
    # emits one table load instead of alternating per chunk (~2.7us each).
    orig_tables = bacc_mod.get_activation_tables

    def one_set(arch):
        t = orig_tables(arch)
        return {k: (v if k == "silu_and_others" else set())
                for k, v in t.items()}

    bacc_mod.get_activation_tables = one_set
    try:
        nc.compile()
    finally:
        bacc_mod.get_activation_tables = orig_tables
    return nc


# ===================== host-side preprocessing =====================

def _shard_edges(ei, t, lu_src, n_dst, n_shards):
    """Split one edge type into per-dst-range shards, dst-sorted."""
    src = ei[0].astype(np.int64)
    dst = ei[1].astype(np.int64)
    rel = (t - lu_src[src]).astype(np.float32)
    order = np.argsort(dst, kind="stable")
    src_s, dst_s, rel_s = src[order], dst[order], rel[order]
    nd_real = n_dst // n_shards
    shards = []
    for ci in range(n_shards):
        r0 = ci * nd_real
        e0, e1 = np.searchsorted(dst_s, [r0, r0 + nd_real])
        shards.append((src_s[e0:e1], (dst_s[e0:e1] - r0), rel_s[e0:e1],
                       order[e0:e1]))
    return shards, nd_real


def _block_counts(dloc, nb):
    return np.bincount(dloc >> 7, minlength=nb)


def _prep_shard(shard, msg_all, nb, ncnt, cbase, tc):
    """Build one core's packed per-edge arrays (permuted block order)."""
    src, dloc_g, rel, orig_idx = shard
    counts = _block_counts(dloc_g, nb)
    perm = np.argsort(-counts, kind="stable")      # iteration i -> orig block
    inv = np.empty_like(perm)
    inv[perm] = np.arange(nb)

    # per-edge placement
    bid = (dloc_g >> 7).astype(np.int64)            # original block
    it = inv[bid]                                   # iteration index
    starts = np.concatenate([[0], np.cumsum(counts)[:-1]])
    pos = np.arange(dloc_g.shape[0], dtype=np.int64) - starts[bid]
    assert np.all(pos < np.asarray(ncnt)[it] * 128)
    col = np.asarray(cbase)[it] + (pos >> 7)
    p = pos & 127
    eidx = col * 128 + p

    TE = tc * 128
    ref = np.unique(src)
    srcr = np.searchsorted(ref, src).astype(np.int32)

    assert tc * 128 < 2**31
    sepf = np.zeros(tc * 128, np.int32)            # flat, chunk-major
    sepf[eidx] = srcr
    dstf = np.repeat(
        (np.arange(nb, dtype=np.int64) * 128), np.asarray(ncnt) * 128
    ).astype(np.int32)
    dstf[eidx] += (dloc_g & 127).astype(np.int32)
    dloc = np.full((128, tc), 200.0, np.float32)
    dloc[p, col] = (dloc_g & 127).astype(np.float32)

    def wrap16(flat):
        assert flat.max() < 2**15, flat.max()
        return np.tile(
            flat.astype(np.int16).reshape(-1, 16).T, (8, 1)
        )                                           # [128, tc*8]

    rel5 = np.zeros((5, TE), np.float32)
    rel5[3:5, :] = 1.0
    rhi = rel.astype(BF16).astype(np.float32)
    rel5[0, eidx] = rhi
    rel5[1, eidx] = rel - rhi
    rel5[2, eidx] = rhi

    msgT = np.zeros((128, TE), BF16)
    msgT[:, eidx] = msg_all[orig_idx].T.astype(BF16)

    return {
        "kvidx": wrap16(sepf),
        "qidx": wrap16(dstf),
        "dloc": dloc,
        "rel5": rel5.astype(BF16),
        "msgT": msgT,
    }, ref, perm


def _pack_weights(Wq, bq, Wk, bk, Wv, bv, We, Ws, bs, W_t, b_t):
    def stack2(Wa, Wb):
        a = Wa.reshape(2, 128, HC)
        b = Wb.reshape(2, 128, HC)
        return np.ascontiguousarray(
            np.concatenate([a, b], axis=2).transpose(1, 0, 2)
        ).astype(BF16)

    wt = (W_t[0] / np.float32(TWO_PI)).astype(np.float32)
    bt = ((b_t + np.float32(math.pi / 2)) / np.float32(TWO_PI)).astype(np.float32)

    whi = wt.astype(BF16).astype(np.float32)
    wlo = wt - whi
    bhi = bt.astype(BF16).astype(np.float32)
    blo = bt - bhi
    wb5 = np.stack([whi, whi, wlo, bhi, blo], axis=0).astype(BF16)

    return {
        "wb5": wb5,
        "kvw": stack2(Wk, Wv),
        "qsw": stack2(Wq, Ws),
        "qsb": np.concatenate([bq, bs + bv])[None, :].astype(BF16),
        "ew": np.ascontiguousarray(
            We.reshape(2, 128, HC).transpose(1, 0, 2)
        ).astype(BF16),
    }


def _consts():
    return {
        "iotaF": np.broadcast_to(
            np.arange(128, dtype=np.float32)[None, :], (128, 128)
        ).astype(BF16).copy(),
        "idmat": np.eye(128, dtype=np.float32).astype(BF16),
        "onesr": np.ones((1, 128), BF16),
    }


def _padT(x, npad):
    o = np.zeros((D_IN, npad), BF16)
    o[:, : x.shape[0]] = x.T.astype(BF16)
    return o


def prepare_all(inputs, n_shards=4):
    N = inputs["x_user"].shape[0]
    w_a = tuple(inputs[f"{n}_a"] for n in
                ("Wq", "bq", "Wk", "bk", "Wv", "bv", "We", "Ws", "bs"))
    w_b = tuple(inputs[f"{n}_b"] for n in
                ("Wq", "bq", "Wk", "bk", "Wv", "bv", "We", "Ws", "bs"))

    shards_a, nd_real = _shard_edges(inputs["edge_index_a"], inputs["t_a"],
                                     inputs["last_update_user"], N, n_shards)
    shards_b, _ = _shard_edges(inputs["edge_index_b"], inputs["t_b"],
                               inputs["last_update_item"], N, n_shards)
    ND = -(-nd_real // 128) * 128
    NB = ND // 128
    shards = shards_a + shards_b

    # per-iteration chunk counts = max over all 8 shards of sorted counts
    ncnt = np.ones(NB, np.int64)
    for sh in shards:
        counts = np.sort(_block_counts(sh[1], NB))[::-1]
        ncnt = np.maximum(ncnt, -(-counts // 128))
    ncnt = [int(x) for x in ncnt]
    cbase = [0]
    for n in ncnt:
        cbase.append(cbase[-1] + n)
    tc = cbase[-1]

    # referenced-src packing: NSP = max over shards
    nsr = max(np.unique(sh[0]).shape[0] for sh in shards)
    NSP = -(-nsr // 128) * 128

    cfg = Cfg(NSP=NSP, ND=ND, NCNT=ncnt)
    consts = _consts()
    wpack_a = _pack_weights(*w_a, inputs["W_t"], inputs["b_t"])
    wpack_b = _pack_weights(*w_b, inputs["W_t"], inputs["b_t"])

    # dst nodes with no incoming edges must not receive the bv fold
    # (reference: empty segment_sum has no bv term) — fixed host-side.
    empties = []
    for typ, shs in (("a", shards_a), ("b", shards_b)):
        bv = inputs[f"bv_{typ}"]
        has_edge = np.zeros(n_shards * nd_real, bool)
        for ci, sh in enumerate(shs):
            has_edge[ci * nd_real + sh[1]] = True
        empties.append((np.nonzero(~has_edge)[0], bv))

    maps, perms = [], []
    for si, sh in enumerate(shards):
        typ_a = si < n_shards
        x_src = inputs["x_user"] if typ_a else inputs["x_item"]
        x_dst = inputs["x_item"] if typ_a else inputs["x_user"]
        msg = inputs["msg_a"] if typ_a else inputs["msg_b"]
        ci = si % n_shards
        r0 = ci * nd_real
        m, ref, perm = _prep_shard(sh, msg, NB, ncnt, cbase, tc)
        m.update(wpack_a if typ_a else wpack_b)
        m.update(consts)
        m["xsT"] = _padT(x_src[ref], NSP)
        xd = np.zeros((ND, D_IN), np.float32)
        xd[:nd_real] = x_dst[r0 : r0 + nd_real]
        xdp = np.zeros((ND, D_IN), np.float32)
        for i in range(NB):
            xdp[i * 128 : (i + 1) * 128] = xd[perm[i] * 128 : (perm[i] + 1) * 128]
        m["xdT"] = np.ascontiguousarray(xdp.T).astype(BF16)
        maps.append(m)
        perms.append(perm)
    return cfg, maps, perms, nd_real, NB, empties


def kernel(**inputs):
    from concourse.bass_utils import run_bass_kernel_spmd

    cfg, in_maps, perms, nd_real, NB, empties = prepare_all(inputs)
    nc = build_program(cfg)
    import os
    trace_env = os.environ.get("BASS_KERNEL_TRACE", "")
    kw = {}
    if trace_env:
        kw = dict(trace=True,
                  trace_cores=[int(c) for c in trace_env.split(",")])
    res = run_bass_kernel_spmd(nc, in_maps, core_ids=list(range(8)), **kw)
    global LAST_RESULTS, LAST_NC
    LAST_RESULTS = res
    LAST_NC = nc

    outs = []
    for si, r in enumerate(res.results):
        o = np.asarray(r["out"]).astype(np.float32)
        perm = perms[si]
        ou = np.empty_like(o)
        for i in range(NB):
            ou[perm[i] * 128 : (perm[i] + 1) * 128] = o[i * 128 : (i + 1) * 128]
        outs.append(ou[:nd_real])
    out_item = np.concatenate(outs[0:4], axis=0)
    out_user = np.concatenate(outs[4:8], axis=0)
    out_item[empties[0][0]] -= empties[0][1][None, :]
    out_user[empties[1][0]] -= empties[1][1][None, :]
    return out_user, out_item

